# revision 1
# baseline (speedup 1.0000x reference)
"""Distributed Trainium2 kernel for the contrastive InfoNCE loss problem.

Strategy: shard the P = SY*SX = 275 position axis across 8 NeuronCores
(36 position slots per core, zero-padded + weight-masked).  All logits
in [m, n] layout (m = positive index on partitions):
    pos/pred projections as bf16 matmuls (two 1-bank PSUM phases), DVE
    adds biases and converts PSUM->SBUF bf16,
    logits1/logits2 share the pos-chunk stationary operand,
    exp on ScalarE as one 1024-wide activation per position,
    LSE column sums via shifted ones-band matmuls accumulating rows of
    a PSUM tile (row j = position j),
    diag2 = sum_n pred*pos via GpSimd product + DVE segmented reduce,
    diag1 via a bf16 running input sum (DVE) + epilogue matmul.
Bias matrices (m/c joins) are precomputed on the host.  Host sums the
per-core partial scalars.
"""

import numpy as np

# Problem constants (from the nn_ALL_9320079032780 spec).
N = 256
C = 128
SY, SX = 11, 25
P = SY * SX  # 275
D = 128
DM = 64
DC = 64
N_CORES = 8
POS_PER_CORE = 36  # padded; 18 supers of 2 positions
N_SUPERS = POS_PER_CORE // 2

EXP_SHIFT = 20.0

WBW = 35 + 128  # width of the shifted ones-column band matrix

# packed bf16 statics layout: [Wl | fT | wband]
_OFF_WL = 0
_OFF_FT = _OFF_WL + D
_OFF_WB = _OFF_FT + N
STB_COLS = _OFF_WB + WBW
# packed f32 statics layout:
# [biasPT | biasTT | fTf | wposb | shift | wrow | ones]
_F_BPB = 0                      # 512: [biasP^T | biasT^T]
_F_FTF = _F_BPB + 2 * N         # 256
_F_WPOS = _F_FTF + N            # 36 (pad mask per position col)
_F_SHIFT = _F_WPOS + POS_PER_CORE
_F_WROW = _F_SHIFT + 1
_F_ONES = _F_WROW + 1
STF_COLS = _F_ONES + 1

_CACHED_NC = None


def _build_nc():
    import concourse.bass as bass  # noqa: F401
    import concourse.mybir as mybir
    import concourse.tile as tile
    from concourse import bacc

    f32 = mybir.dt.float32
    bf16 = mybir.dt.bfloat16
    Alu = mybir.AluOpType
    Act = mybir.ActivationFunctionType

    nc = bacc.Bacc("TRN2", target_bir_lowering=False, debug=False,
                   num_devices=N_CORES)

    # Make the act-table pass pick the combined exp+ln set so the kernel
    # pays a single ACT_TABLE_LOAD instead of one per function family.
    from concourse.hw_specs import get_activation_tables
    _tabs = get_activation_tables(nc.m.arch)
    _Exp, _Ln = mybir.ActivationFunctionType.Exp, mybir.ActivationFunctionType.Ln
    for _name, _fns in _tabs.items():
        if _name != "natural_log_exp_and_others":
            _fns.discard(_Exp)
            _fns.discard(_Ln)

    loc_d = nc.declare_dram_parameter("loc", [N_SUPERS, C, 4 * N], bf16, isOutput=False)
    stb_d = nc.declare_dram_parameter("stb", [128, STB_COLS], bf16, isOutput=False)
    stf_d = nc.declare_dram_parameter("stf", [128, STF_COLS], f32, isOutput=False)
    out_d = nc.declare_dram_parameter("out", [1, 4], f32, isOutput=True)

    with tile.TileContext(nc) as tc:
        with (
            tc.tile_pool(name="statics", bufs=1) as st,
            tc.tile_pool(name="loc", bufs=4) as locpool,
            tc.tile_pool(name="ptp", bufs=2) as work,
            tc.tile_pool(name="d2", bufs=3) as work2,
            tc.tile_pool(name="exps", bufs=6) as exps,
            tc.tile_pool(name="pp", bufs=1, space="PSUM") as ppp,
            tc.tile_pool(name="lg", bufs=2, space="PSUM") as lgp,
            tc.tile_pool(name="sg", bufs=1, space="PSUM") as sgp,
        ):
            # ---- statics: two packed DMAs ----
            stb = st.tile([128, STB_COLS], bf16, tag="stb")
            stf = st.tile([128, STF_COLS], f32, tag="stf")
            nc.sync.dma_start(out=stb[:, :], in_=stb_d[:, :])
            nc.scalar.dma_start(out=stf[:, :], in_=stf_d[:, :])
            Wl = stb[:, _OFF_WL:_OFF_WL + D]
            fT = stb[:, _OFF_FT:_OFF_FT + N]
            wband = stb[:, _OFF_WB:_OFF_WB + WBW]
            biasPB = stf[:, _F_BPB:_F_BPB + 2 * N]
            fTf = stf[:, _F_FTF:_F_FTF + N]
            wposb = stf[:, _F_WPOS:_F_WPOS + POS_PER_CORE]
            shiftc = stf[:, _F_SHIFT:_F_SHIFT + 1]
            wrow = stf[:, _F_WROW:_F_WROW + 1]
            onesf = stf[:, _F_ONES:_F_ONES + 1]
            biasPB4 = (biasPB.rearrange("p (b n) -> p b n", b=2)
                       .unsqueeze(2).broadcast_to([128, 2, 2, N]))

            # HAM warmup: dummy matmuls on a memset tile keep the PE busy
            # through the cold window while the first DMAs land.
            wtile = st.tile([128, 128], bf16, tag="wtile")
            nc.vector.memset(wtile[:, :], 0.0)
            wps = lgp.tile([128, 4 * N], f32, tag="lg")
            for _w in range(5):
                nc.tensor.matmul(
                    out=wps[:, 0:2 * N].rearrange("p (k n) -> p k n", k=4),
                    lhsT=wtile,
                    rhs=wtile[:, :].unsqueeze(1).broadcast_to([128, 4, 128]),
                    start=True, stop=True)

            # persistent accumulators
            # S rows j: [lse1 sums (256) | lse2 sums (256)] for position j
            S = sgp.tile([128, 2 * N], f32, tag="S")
            # SD rows j: per-n diag2 partition sums for position j
            SD = sgp.tile([128, 2 * N], f32, tag="SD")
            # running bf16 input sum for diag1 (folded in the epilogue)
            lsum = st.tile([128, 2 * N], bf16, tag="lsum")
            nc.vector.memset(lsum[:, :], 0.0)

            def stage_dma(s):
                lpt = locpool.tile([C, 4 * N], bf16, tag="lpt")
                q = nc.sync if s > 0 else nc.scalar
                q.dma_start(out=lpt[:, :], in_=loc_d[s, :, :])
                return lpt

            def stage_proj(s, lpt):
                pp = ppp.tile([128, 4 * N], f32, tag="pp")
                nc.tensor.matmul(out=pp[:, 0:2 * N], lhsT=Wl,
                                 rhs=lpt[:, 0:2 * N], start=True, stop=True)
                nc.tensor.matmul(out=pp[:, 2 * N:4 * N], lhsT=Wl,
                                 rhs=lpt[:, 2 * N:4 * N], start=True, stop=True)
                ptp = work.tile([128, 4 * N], bf16, tag="ptp")
                nc.vector.tensor_tensor(
                    out=ptp[:, :].rearrange("p (b k n) -> p b k n", b=2, k=2),
                    in0=pp[:, :].rearrange("p (b k n) -> p b k n", b=2, k=2),
                    in1=biasPB4, op=Alu.add)
                nc.vector.tensor_tensor(out=lsum[:, :], in0=lsum[:, :],
                                        in1=lpt[:, 0:2 * N], op=Alu.add)
                return ptp

            def emit_pos_logits(ptp, k):
                # [m, n] layout for position k of the super: quarters
                # (h0:l1, h0:l2, h1:l1, h1:l2); l1/l2 share the pos-chunk
                # stationary operand.
                lg = lgp.tile([128, 4 * N], f32, tag="lg")
                predp = ptp[:, 2 * N + k * N:2 * N + (k + 1) * N]
                for h in range(2):
                    ch = ptp[:, k * N + h * 128:k * N + (h + 1) * 128]
                    nc.tensor.matmul(out=lg[:, 2 * h * N:(2 * h + 1) * N],
                                     lhsT=ch, rhs=fT, start=True, stop=True)
                    nc.tensor.matmul(out=lg[:, (2 * h + 1) * N:(2 * h + 2) * N],
                                     lhsT=ch, rhs=predp, start=True, stop=True)
                et = exps.tile([128, 4 * N], bf16, tag="et")
                # exp(l - EXP_SHIFT); the shift is added back on the host.
                nc.scalar.activation(et[:, :], lg[:, :], Act.Exp,
                                     bias=shiftc[:, 0:1])
                return et

            def emit_lse(j, et, h):
                # shifted ones-column band: lhsT = wband[:, 35-j:163-j] has
                # its all-ones column at output row j.
                nc.tensor.matmul(
                    out=S[:, :],
                    lhsT=wband[:, 35 - j:35 - j + 128],
                    rhs=et[:, h * 2 * N:(h + 1) * 2 * N],
                    start=(j == 0 and h == 0),
                    stop=(j == POS_PER_CORE - 1 and h == 1))

            def emit_d2(s, ptp):
                d2s = work2.tile([128, 2 * N], bf16, tag="d2s")
                nc.gpsimd.tensor_tensor(out=d2s[:, :], in0=ptp[:, 0:2 * N],
                                        in1=ptp[:, 2 * N:4 * N], op=Alu.mult)
                return d2s

            def emit_d2band(s, d2s):
                for k in range(2):
                    j = 2 * s + k
                    nc.tensor.matmul(
                        out=SD[:, 0:N],
                        lhsT=wband[:, 35 - j:35 - j + 128],
                        rhs=d2s[:, k * N:(k + 1) * N],
                        start=(j == 0), stop=(j == POS_PER_CORE - 1))

            # ---- main loop (band matmuls trail the exps by LAG positions)
            LAG = 2
            pend = []
            pend_d2 = []
            lpt = stage_dma(0)
            lpt_nxt = stage_dma(1)
            ptp = stage_proj(0, lpt)
            for s in range(N_SUPERS):
                for k in range(2):
                    et = emit_pos_logits(ptp, k)
                    pend.append((2 * s + k, et))
                d2s = emit_d2(s, ptp)
                pend_d2.append((s, d2s))

                if s + 1 < N_SUPERS:
                    nlpt = lpt_nxt
                    if s + 2 < N_SUPERS:
                        lpt_nxt = stage_dma(s + 2)
                    ptp = stage_proj(s + 1, nlpt)
                while len(pend) > LAG:
                    j, et = pend.pop(0)
                    emit_lse(j, et, 0)
                    emit_lse(j, et, 1)
                while len(pend_d2) > 2:
                    emit_d2band(*pend_d2.pop(0))

            # diag epilogue pieces that can overlap the trailing bands
            while pend_d2:
                emit_d2band(*pend_d2.pop(0))
            sdred = st.tile([128, 1], f32, tag="sdred")
            nc.vector.tensor_reduce(out=sdred[0:POS_PER_CORE, 0:1],
                                    in_=SD[0:POS_PER_CORE, 0:N],
                                    axis=mybir.AxisListType.X, op=Alu.add)
            lsumf = st.tile([128, N], bf16, tag="lsumf")
            nc.vector.tensor_tensor(
                out=lsumf[:, :], in0=lsum[:, 0:N], in1=lsum[:, N:2 * N],
                op=Alu.add)
            gp = ppp.tile([128, 4 * N], f32, tag="pp")
            nc.tensor.matmul(out=gp[:, 0:N], lhsT=Wl, rhs=lsumf[:, :],
                             start=True, stop=True)
            scr = st.tile([128, N], f32, tag="scr")
            scol = st.tile([128, 1], f32, tag="scol")
            nc.vector.tensor_tensor(out=scr[:, :], in0=gp[:, 0:N],
                                    in1=fTf, op=Alu.mult)
            nc.vector.tensor_reduce(out=scol[:, 0:1], in_=scr[:, :],
                                    axis=mybir.AxisListType.X, op=Alu.add)
            psF = lgp.tile([128, 4 * N], f32, tag="lg")
            nc.tensor.matmul(out=psF[0:1, 2:3], lhsT=onesf,
                             rhs=scol[:, :], start=True, stop=True)
            nc.tensor.matmul(out=psF[0:1, 1:2],
                             lhsT=wrow[0:POS_PER_CORE, :],
                             rhs=sdred[0:POS_PER_CORE, :],
                             start=True, stop=True)

            while pend:
                j, et = pend.pop(0)
                emit_lse(j, et, 0)
                emit_lse(j, et, 1)

            # ---- tail: ln over lse sums, masked row total, pack output
            J = POS_PER_CORE
            lnS = st.tile([128, 2 * N], f32, tag="lnS")
            logacc = st.tile([128, 1], f32, tag="logacc")
            nc.scalar.activation(lnS[0:J, :], S[0:J, :], Act.Ln,
                                 accum_out=logacc[0:J, 0:1])
            nc.tensor.matmul(out=psF[0:1, 0:1], lhsT=wrow[0:J, :],
                             rhs=logacc[0:J, :], start=True, stop=True)
            out_sb = st.tile([1, 4], f32, tag="out_sb")
            nc.vector.memset(out_sb[0:1, :], 0.0)
            nc.vector.tensor_copy(out=out_sb[0:1, 0:3], in_=psF[0:1, 0:3])
            nc.sync.dma_start(out=out_d[:, :], in_=out_sb[0:1, :])

    nc.finalize()
    return nc


def _get_nc():
    global _CACHED_NC
    if _CACHED_NC is None:
        _CACHED_NC = _build_nc()
    return _CACHED_NC


def _core_position_lists():
    """275 positions -> 8 cores: 3 cores x 35, 5 cores x 34."""
    lists = []
    start = 0
    for i in range(N_CORES):
        cnt = 35 if i < 3 else 34
        lists.append(list(range(start, start + cnt)))
        start += cnt
    assert start == P
    return lists


def _prep_in_maps(f_t_global, x_t_local, x_t_prev_local, m_t, m_t_prev, c_t,
                  c_t_prev, W_join, b_join):
    import ml_dtypes
    bf16 = ml_dtypes.bfloat16

    # [N, C, SY, SX] -> [P, C, N]
    locp_full = np.ascontiguousarray(
        x_t_prev_local.reshape(N, C, P).transpose(2, 1, 0))
    loct_full = np.ascontiguousarray(
        x_t_local.reshape(N, C, P).transpose(2, 1, 0))

    Wm = W_join[C:C + DM]
    Wc = W_join[C + DM:]
    biasP = m_t_prev @ Wm + c_t_prev @ Wc + b_join  # [N, D]
    biasT = m_t @ Wm + c_t @ Wc + b_join            # [N, D]

    # packed bf16 statics [128, STB_COLS]
    stb = np.zeros((128, STB_COLS), dtype=np.float32)
    stb[:, _OFF_WL:_OFF_WL + D] = W_join[:C]
    stb[:, _OFF_FT:_OFF_FT + N] = f_t_global.T
    stb[:, _OFF_WB + 35] = 1.0
    stb = stb.astype(bf16)

    in_maps = []
    for ids in _core_position_lists():
        npos = len(ids)
        locp = np.zeros((POS_PER_CORE, C, N), dtype=np.float32)
        loct = np.zeros((POS_PER_CORE, C, N), dtype=np.float32)
        locp[:npos] = locp_full[ids]
        loct[:npos] = loct_full[ids]
        # [36, C, N] -> [18, C, 2N] (two consecutive positions side by side)
        locp = np.ascontiguousarray(
            locp.reshape(N_SUPERS, 2, C, N).transpose(0, 2, 1, 3)
                .reshape(N_SUPERS, C, 2 * N))
        loct = np.ascontiguousarray(
            loct.reshape(N_SUPERS, 2, C, N).transpose(0, 2, 1, 3)
                .reshape(N_SUPERS, C, 2 * N))
        loc = np.concatenate([locp, loct], axis=2).astype(bf16)
        # packed f32 statics [128, STF_COLS]
        stf = np.zeros((128, STF_COLS), dtype=np.float32)
        stf[:, _F_BPB:_F_BPB + N] = biasP.T
        stf[:, _F_BPB + N:_F_BPB + 2 * N] = biasT.T
        stf[:, _F_FTF:_F_FTF + N] = f_t_global.T
        for j in range(npos):
            stf[:, _F_WPOS + j] = 1.0
        stf[:, _F_SHIFT] = -EXP_SHIFT
        stf[0:npos, _F_WROW] = 1.0
        stf[:, _F_ONES] = 1.0
        in_maps.append({"loc": loc, "stb": stb, "stf": stf})
    return in_maps


def kernel(f_t_global, x_t_local, x_t_prev_local, m_t, m_t_prev, c_t,
           c_t_prev, W_join, b_join):
    from concourse.bass_utils import run_bass_kernel_spmd

    args = [f_t_global, x_t_local, x_t_prev_local, m_t, m_t_prev, c_t,
            c_t_prev, W_join, b_join]
    args = [np.asarray(a, dtype=np.float32) for a in args]
    (f_t_global, x_t_local, x_t_prev_local, m_t, m_t_prev, c_t,
     c_t_prev, W_join, b_join) = args
    in_maps = _prep_in_maps(*args)
    nc = _get_nc()
    res = run_bass_kernel_spmd(nc, in_maps, core_ids=list(range(N_CORES)))
    return combine(res, f_t_global, m_t_prev, c_t_prev, W_join, b_join)


def combine(res, f_t_global, m_t_prev, c_t_prev, W_join, b_join):
    """Host-side reduction of the 8 per-core [1, 4] partials."""
    Wm = W_join[C:C + DM]
    Wc = W_join[C + DM:]
    biasP = m_t_prev @ Wm + c_t_prev @ Wc + b_join
    h1 = float(np.sum(f_t_global * biasP))
    total = 0.0
    for i, ids in enumerate(_core_position_lists()):
        v = res.results[i]["out"][0]
        npos = len(ids)
        total += float(v[0]) - float(v[1]) - float(v[2]) - npos * h1
    return np.asarray(total / (P * N) + 2.0 * EXP_SHIFT, dtype=np.float32)

